# revision 20
# baseline (speedup 1.0000x reference)
"""Trainium2 Bass kernel for nn_Encoding_layer (highway stack + pairwise MLP
attention + fuse gates).

Sharding: data-parallel over batch B=16 across 8 NeuronCores (2 batches per
core); all dense weights replicated. No collectives.

v3 design (mostly-fp8 matmul path):
  - fp8e4 DoubleRow GEMMs (256-contraction/instr): highway layers 0 and 1,
    attention scores/numerator/denominator, z-gate, r-gate att-half.
    fp8 weights are scaled x16 before the cast (raw 0.02-scale weights sit
    in e4m3's subnormal range); the 1/16 folds into the drain scale.
  - The r-gate inputs-half stays bf16: its error reaches the output
    un-smoothed (r multiplies raw inputs, |x0| up to ~4.5).  Attention-side
    noise only reaches the output through gate logits (~0.02 weights), so
    phase-D/x1/x2 fp8 noise is diluted ~20x.
  - s1 and s2 share one matmul (lhsT has a w1 column and a w2 column);
    the s2 row is scattered to partitions with contraction-2 matmuls
    selecting row 1 via an identity column.
  - Zero-bias fast path: tb/ffb/frb checked at runtime; fast variant drops
    phase-E bias matmuls and uses vector-relu drains.  A bias-capable
    variant compiles on demand.
  - Phase D: per unit, all 8 score matmuls are emitted before the
    numerator accumulations so the in-order PE queue never waits on the
    exp/clamp chain.  th/1-per-unit reciprocal broadcasts ride idle DMA
    engines (partition-stride-0 source APs) instead of PE matmuls; pn
    psum banks drain via a fused multiply (psum x 1/den -> fp8 attT8)
    deferred to the start of the next unit.

Per-core layouts (n = 2 batches x L=1024 = 2048 token-columns):
  xT8/x1T8/x2T8/w3x8/attT8 (fp8), xTb (bf16): [128, 4, 2048]
    [u mod 128, u div 128, n]
  xO8 (fp8): [128, 16, 512] row-major x2 (lhsT for the attention numerator)
  Attention: S^T[j,i] = s3[j,i] (PE, w3*x^T as lhsT) + s2[j] (ACT exp bias).
  The per-column term s1[i]+ab never enters the matmuls: a per-column
  factor cancels in the softmax, so relu becomes a clamp against
  th[i] = exp(-(s1[i]+ab)).
"""

import numpy as np

B, L, U, H = 16, 1024, 512, 2
NCORES = 8
BPC = B // NCORES          # batches per core
N = BPC * L                # token columns per core
KU = U // 128              # 4  u-tiles
NT = N // 128              # 16 row-tiles per core
NS = N // 512              # 4  512-wide column slices per core
JT = L // 128              # 8  j-tiles per batch
IH = L // 512              # 2  i-halves per batch
WS = 16.0                  # fp8 weight scale
IWS = 1.0 / WS


def build_nc(with_bias: bool):
    import concourse.bacc as bacc
    import concourse.tile as tile
    from concourse import mybir
    from concourse.masks import make_identity

    F32 = mybir.dt.float32
    BF16 = mybir.dt.bfloat16
    FP8 = mybir.dt.float8e4
    AF = mybir.ActivationFunctionType
    OP = mybir.AluOpType
    DR = mybir.MatmulPerfMode.DoubleRow

    nc = bacc.Bacc("TRN2", target_bir_lowering=False, debug=False,
                   num_devices=NCORES)

    x_in = nc.dram_tensor("inputs", [BPC, L, U], F32, kind="ExternalInput").ap()
    tW = nc.dram_tensor("tW", [H, U, U], F32, kind="ExternalInput").ap()
    tb = nc.dram_tensor("tb", [H, U], F32, kind="ExternalInput").ap()
    cW = nc.dram_tensor("cW", [H, U, U], F32, kind="ExternalInput").ap()
    cb = nc.dram_tensor("cb", [H, U], F32, kind="ExternalInput").ap()
    aW = nc.dram_tensor("aW", [3 * U], F32, kind="ExternalInput").ap()
    ab = nc.dram_tensor("ab", [1], F32, kind="ExternalInput").ap()
    frW = nc.dram_tensor("frW", [2 * U, U], F32, kind="ExternalInput").ap()
    frb = nc.dram_tensor("frb", [U], F32, kind="ExternalInput").ap()
    ffW = nc.dram_tensor("ffW", [2 * U, U], F32, kind="ExternalInput").ap()
    ffb = nc.dram_tensor("ffb", [U], F32, kind="ExternalInput").ap()
    out = nc.dram_tensor("out", [BPC, L, U], F32, kind="ExternalOutput").ap()

    xv = x_in.flatten_outer_dims().rearrange("(t p) u -> t p u", p=128)
    outv = out.flatten_outer_dims().rearrange("(t p) u -> t p u", p=128)

    fWv = ffW.rearrange("(k p) m -> k p m", p=128)
    rWv = frW.rearrange("(k p) m -> k p m", p=128)

    with tile.TileContext(nc) as tc:
        with tc.tile_pool(name="pers", bufs=1) as pers, \
             tc.tile_pool(name="dram", bufs=1, space="DRAM") as dpool:
            # ---- persistent SBUF tensors ----
            xT8 = pers.tile([128, KU, N], FP8, tag="xT8")      # inputs^T
            xTb = pers.tile([128, KU, N], BF16, tag="xTb")     # inputs^T bf16
            x1T8 = pers.tile([128, KU, N], FP8, tag="x1T8")
            x2T8 = pers.tile([128, KU, N], FP8, tag="x2T8")
            w3x8 = pers.tile([128, KU, N], FP8, tag="w3x8")
            attT8 = pers.tile([128, KU, N], FP8, tag="attT8")
            xO8 = pers.tile([128, NT, U], FP8, tag="xO8")
            tW8 = pers.tile([128, KU, U], FP8, tag="tW8")      # layer0, x16
            cW8 = pers.tile([128, KU, U], FP8, tag="cW8")      # layer0, x16
            tW8b = pers.tile([128, KU, U], FP8, tag="tW8b")    # layer1, x16
            cW8b = pers.tile([128, KU, U], FP8, tag="cW8b")    # layer1, x16
            fW8 = pers.tile([128, 2 * KU, U], FP8, tag="fW8")  # x16
            rWb16 = pers.tile([128, KU, U], BF16, tag="rWb16")  # x16 inputs
            rW8f = pers.tile([128, KU, U], FP8, tag="rW8f")    # x16 att-half
            tbsb = pers.tile([128, H, KU], F32, tag="tbsb")
            cbsb = pers.tile([128, H, KU], F32, tag="cbsb")
            awsb = pers.tile([128, 12], F32, tag="awsb")       # w1|w2|w3 cols
            aw3 = pers.tile([128, KU], F32, tag="aw3")         # 16*w3
            w12h8 = pers.tile([128, KU, 16], FP8, tag="w12h8")  # x16 w1,w2,0.
            ab_sb = pers.tile([1, 1], F32, tag="ab_sb")
            nab_sb = pers.tile([1, 1], F32, tag="nab_sb")
            ffb16 = pers.tile([1, U], BF16, tag="ffb16")       # x16
            frb16 = pers.tile([1, U], BF16, tag="frb16")       # x16
            thr = pers.tile([1, N], BF16, tag="thr")           # exp(-(s1+ab))
            thrd = dpool.tile([1, N], BF16, tag="thrd")        # DRAM copy
            s2f = pers.tile([128, NT], F32, tag="s2f")
            s12sb = pers.tile([2, N], BF16, tag="s12sb")
            ones_row = pers.tile([1, 128], BF16, tag="ones_row")
            ones216 = pers.tile([128, 2, 16], FP8, tag="ones216")
            identf = pers.tile([128, 128], F32, tag="identf")
            ident8 = pers.tile([128, 128], FP8, tag="ident8")
            identb = pers.tile([128, 128], BF16, tag="identb")

            nc.vector.memset(ones_row, 1.0)
            nc.vector.memset(ones216, 1.0)
            make_identity(nc, identf)
            make_identity(nc, ident8)
            make_identity(nc, identb)

            # fuse-gate weight chunks dripped through phases B+C and D
            fuse_chunks = (
                [(fWv, fW8, k, k) for k in range(2 * KU)] +
                [(rWv, rWb16, k, k) for k in range(KU)] +
                [(rWv, rW8f, k, k - KU) for k in range(KU, 2 * KU)])

            def emit_fuse(ci):
                wv_, wdst_, ksrc_, kdst_ = fuse_chunks[ci]
                wsf = pers.tile([128, U], F32, tag="wsf", bufs=4,
                                name=f"wsf_{ci}")
                nc.sync.dma_start(wsf, wv_[ksrc_])
                if ci % 2 == 0:
                    nc.vector.tensor_scalar_mul(wdst_[:, kdst_, :], wsf, WS)
                else:
                    nc.scalar.mul(wdst_[:, kdst_, :], wsf, WS)

            # ======== Phase A: loads, transpose, highway layers ==========
            with tc.tile_pool(name="stg", bufs=3) as stg, \
                 tc.tile_pool(name="stgx", bufs=8) as stgx, \
                 tc.tile_pool(name="stgw", bufs=8) as stgw, \
                 tc.tile_pool(name="transP", bufs=2, space="PSUM") as transP, \
                 tc.tile_pool(name="hw0P", bufs=3, space="PSUM") as hw0P:
                def emit_weights(l, wi):
                    wsrc = (tW, cW)[wi]
                    wdst = ((tW8, cW8), (tW8b, cW8b))[l][wi]
                    wv = wsrc[l].rearrange("(k p) m -> k p m", p=128)
                    for k in range(KU):
                        ws = stgw.tile([128, U], F32, tag="ws",
                                       name=f"ws_{l}_{wi}_{k}")
                        nc.sync.dma_start(ws, wv[k])
                        if k % 2 == 0:
                            nc.vector.tensor_scalar_mul(wdst[:, k, :], ws, WS)
                        else:
                            nc.scalar.mul(wdst[:, k, :], ws, WS)

                # small tensors first (layer-0 needs biases)
                nc.sync.dma_start(
                    tbsb, tb.rearrange("l (m p) -> p l m", p=128))
                nc.sync.dma_start(
                    cbsb, cb.rearrange("l (m p) -> p l m", p=128))
                nc.sync.dma_start(
                    awsb, aW.rearrange("(w m p) -> p (w m)", p=128, w=3))
                nc.vector.memset(w12h8, 0.0)
                nc.vector.tensor_scalar_mul(w12h8[:, :, 0], awsb[:, 0:KU], WS)
                nc.vector.tensor_scalar_mul(
                    w12h8[:, :, 1], awsb[:, KU:2 * KU], WS)
                nc.vector.tensor_scalar_mul(aw3, awsb[:, 2 * KU:3 * KU], WS)
                nc.sync.dma_start(ab_sb, ab[None, :])
                nc.scalar.mul(nab_sb, ab_sb, -1.0)
                fb = stg.tile([1, U], F32, tag="fb")
                nc.sync.dma_start(fb, ffb[None, :])
                nc.scalar.mul(ffb16, fb, WS)
                fb2 = stg.tile([1, U], F32, tag="fb")
                nc.sync.dma_start(fb2, frb[None, :])
                nc.scalar.mul(frb16, fb2, WS)

                # warm the PE p-state during the initial DMA wait
                wpt = hw0P.tile([128, 512], F32, tag="pt", name="warm_pt")
                wpc = hw0P.tile([128, 512], F32, tag="pc", name="warm_pc")
                for i in range(24):
                    nc.tensor.matmul((wpt, wpc)[i % 2][:, 0:128],
                                     identf, identf,
                                     start=True, stop=True)

                def trans_block(tg, tt):
                    t = tg * 4 + tt
                    xs = stgx.tile([128, U], F32, tag="xs",
                                   name=f"xs_{t}")
                    nc.sync.dma_start(xs, xv[t])
                    ptt = transP.tile([128, 512], F32, tag="ptt",
                                      name=f"ptt_{t}")
                    for k in range(KU):
                        nc.tensor.transpose(
                            ptt[:, k * 128:(k + 1) * 128],
                            xs[:, k * 128:(k + 1) * 128], identf)
                    csl = slice(tg * 512 + tt * 128,
                                tg * 512 + (tt + 1) * 128)
                    pv = ptt.rearrange("p (k c) -> p k c", k=KU)
                    nc.scalar.copy(xTb[:, :, csl], pv)

                def trans_group(tg):
                    for tt in range(4):
                        trans_block(tg, tt)
                    nsl = slice(tg * 512, (tg + 1) * 512)
                    nc.vector.tensor_copy(xT8[:, :, nsl], xTb[:, :, nsl])

                def hw_layer(l, tg):
                    nsl = slice(tg * 512, (tg + 1) * 512)
                    xmm = xT8 if l == 0 else x1T8
                    xew = xTb[:, :, nsl] if l == 0 else x1T8[:, :, nsl]
                    wT, wC = ((tW8, cW8), (tW8b, cW8b))[l]
                    th = stg.tile([128, KU, 512], BF16, tag="th",
                                  name=f"th{l}_{tg}")
                    ch = stg.tile([128, KU, 512], BF16, tag="ch",
                                  name=f"ch{l}_{tg}")
                    for m in range(KU):
                        pt = hw0P.tile([128, 512], F32, tag="pt",
                                       name=f"pt{l}_{tg}_{m}")
                        pc = hw0P.tile([128, 512], F32, tag="pc",
                                       name=f"pc{l}_{tg}_{m}")
                        for kk in range(2):
                            ksl = slice(2 * kk, 2 * kk + 2)
                            nc.tensor.matmul(
                                pt, wT[:, ksl, m * 128:(m + 1) * 128],
                                xmm[:, ksl, nsl],
                                start=(kk == 0), stop=(kk == 1),
                                perf_mode=DR)
                        for kk in range(2):
                            ksl = slice(2 * kk, 2 * kk + 2)
                            nc.tensor.matmul(
                                pc, wC[:, ksl, m * 128:(m + 1) * 128],
                                xmm[:, ksl, nsl],
                                start=(kk == 0), stop=(kk == 1),
                                perf_mode=DR)
                        # drains: relu vector (m 0,1) / scalar (m 2,3);
                        # sigmoid always on scalar (ACT-only func)
                        if with_bias or m >= 2:
                            nc.scalar.activation(
                                th[:, m, :], pt, AF.Relu,
                                bias=tbsb[:, l, m:m + 1], scale=IWS)
                        else:
                            nc.vector.tensor_scalar(
                                th[:, m, :], pt, 0.0, IWS,
                                op0=OP.max, op1=OP.mult)
                        nc.scalar.activation(
                            ch[:, m, :], pc, AF.Sigmoid,
                            bias=cbsb[:, l, m:m + 1], scale=IWS)
                    # slice-wide elementwise: x_out = xin + c*(t - xin)
                    dh = stg.tile([128, KU, 512], BF16, tag="dh",
                                  name=f"dh{l}_{tg}")
                    nc.vector.tensor_tensor(dh, th, xew, op=OP.subtract)
                    mh = stg.tile([128, KU, 512], BF16, tag="mh",
                                  name=f"mh{l}_{tg}")
                    nc.vector.tensor_tensor(mh[:, 0:3, :], ch[:, 0:3, :],
                                            dh[:, 0:3, :], op=OP.mult)
                    nc.gpsimd.tensor_tensor(mh[:, 3, :], ch[:, 3, :],
                                            dh[:, 3, :], op=OP.mult)
                    dst = x1T8 if l == 0 else x2T8
                    nc.vector.tensor_tensor(
                        dst[:, :, nsl], xew, mh, op=OP.add)

                trans_group(0)
                emit_weights(0, 0)
                emit_weights(0, 1)

                # 3-stream pipeline: T(s) | L0(s-1) | L1(s-2)
                for s in range(1, NS + 2):
                    if s - 1 < NS:
                        hw_layer(0, s - 1)
                    if s < NS:
                        trans_group(s)
                    if 0 <= s - 2:
                        hw_layer(1, s - 2)
                    if s == 1:
                        emit_weights(1, 0)
                        emit_weights(1, 1)

            # ===== Phase B+C: attention prep (xO8 transposes, w3x, s1,
            # s2 via merged matmul + partition scatter) ============
            with tc.tile_pool(name="xop", bufs=2, space="PSUM") as xop, \
                 tc.tile_pool(name="pc1", bufs=1, space="PSUM") as pc1:
                ps12 = pc1.tile([16, 512], F32, tag="ps12")
                s2p = pc1.tile([128, NT], F32, tag="s2p")

                def prep_block(t):
                    nsl = slice(t * 512, (t + 1) * 512)
                    for k in range(KU):
                        nc.vector.tensor_scalar_mul(
                            w3x8[:, k, nsl], x2T8[:, k, nsl],
                            aw3[:, k:k + 1])
                    for jt in range(4 * t, 4 * t + 4):
                        ptr = xop.tile([128, 512, 2], FP8, tag="ptr",
                                       name=f"ptr_{jt}")
                        for k in range(KU):
                            nc.tensor.transpose(
                                ptr[:, k * 128:(k + 1) * 128, 0],
                                x2T8[:, k, jt * 128:(jt + 1) * 128],
                                ident8)
                        if jt % 2 == 0:
                            nc.vector.tensor_copy(
                                xO8[:, jt, :], ptr[:, :, 0])
                        else:
                            nc.scalar.copy(xO8[:, jt, :], ptr[:, :, 0])
                    # merged s1/s2: out row0 = x2.w1, row1 = x2.w2
                    for kk in range(2):
                        ksl = slice(2 * kk, 2 * kk + 2)
                        nc.tensor.matmul(ps12, w12h8[:, ksl, :],
                                         x2T8[:, ksl, nsl],
                                         start=(kk == 0), stop=(kk == 1),
                                         perf_mode=DR)
                    nc.scalar.activation(
                        thr[:, nsl], ps12[0:1, :], AF.Exp,
                        bias=nab_sb, scale=-IWS)
                    nc.sync.dma_start(thrd[:, nsl], thr[:, nsl])
                    nc.vector.tensor_copy(s12sb[:, nsl], ps12[0:2, :])
                    # scatter s2 row chunks onto partitions: contract-2
                    # matmul whose rhs (identity col 1) selects row 1
                    for jt in range(4 * t, 4 * t + 4):
                        nc.tensor.matmul(
                            s2p[:, jt:jt + 1],
                            s12sb[:, jt * 128:(jt + 1) * 128],
                            identb[0:2, 1:2], start=True, stop=True)
                    emit_fuse(2 * t)
                    emit_fuse(2 * t + 1)

                for t in range(NS):
                    prep_block(t)
                nc.scalar.mul(s2f, s2p, IWS)

            # ============= Phase D: pairwise softmax attention =============
            with tc.tile_pool(name="pdn", bufs=1, space="PSUM") as pdn, \
                 tc.tile_pool(name="pds", bufs=3, space="PSUM") as pds, \
                 tc.tile_pool(name="prp", bufs=1, space="PSUM") as prp, \
                 tc.tile_pool(name="dsb", bufs=4) as dsb:
                # th-clamp broadcasts ride idle DMA engines, prefetched
                # for all 8 units up front (partition-stride-0 source)
                thbcs = []
                for b in range(BPC):
                    for h in range(IH):
                        isl = slice(b * L + h * 512, b * L + (h + 1) * 512)
                        thbc = dsb.tile([128, 512], BF16, tag="thbc",
                                        bufs=8, name=f"thbc_{b}_{h}")
                        nc.sync.dma_start(
                            thbc, thrd[0:1, isl].broadcast_to([128, 512]))
                        thbcs.append(thbc)

                def make_lastnum(b, h, isl, pn, pr16, emit_num):
                    # last numerator group + psum drain + reciprocal,
                    # deferred into the NEXT unit so the in-order PE
                    # queue never waits on this unit's final exp/clamp
                    def lastnum():
                        emit_num(JT // 2 - 1)
                        pnh = [dsb.tile([128, 512], BF16, tag="pnh",
                                        bufs=8, name=f"pnh_{b}_{h}_{du}")
                               for du in range(KU)]
                        nc.scalar.copy(pnh[0], pn[0])
                        for du in range(1, KU):
                            nc.vector.tensor_copy(pnh[du], pn[du])
                        rec = dsb.tile([1, 512], F32, tag="rec",
                                       name=f"rec_{b}_{h}")
                        nc.vector.reciprocal_approx_fast(rec, pr16[0:1, :])
                        recd = dpool.tile([1, 512], F32, tag="recd",
                                          bufs=2, name=f"recd_{b}_{h}")
                        nc.sync.dma_start(recd, rec)
                        rbc = dsb.tile([128, 512], F32, tag="rbc",
                                       bufs=2, name=f"rbc_{b}_{h}")
                        nc.sync.dma_start(
                            rbc, recd[0:1, :].broadcast_to([128, 512]))

                        def mults():
                            for du in range(KU):
                                nc.vector.tensor_tensor(
                                    attT8[:, du, isl], pnh[du], rbc,
                                    op=OP.mult)
                        return mults
                    return lastnum

                last_deferred = None
                mults_deferred = None
                for b in range(BPC):
                    for h in range(IH):
                        unit = b * IH + h
                        thbc = thbcs[unit]
                        isl = slice(b * L + h * 512, b * L + (h + 1) * 512)
                        pn = [pdn.tile([128, 512], F32, tag=f"pn{du}",
                                       name=f"pn_{b}_{h}_{du}")
                              for du in range(KU)]
                        pr16 = prp.tile([16, 512], F32, tag="pr16",
                                        name=f"pr16_{b}_{h}")
                        eh2s = [dsb.tile([128, 2, 512], FP8, tag="eh2",
                                         bufs=8, name=f"eh2_{b}_{h}_{jp}")
                                for jp in range(JT // 2)]
                        eb2s = [dsb.tile([128, 2, 512], BF16, tag="eb2",
                                         bufs=3, name=f"eb2_{b}_{h}_{jp}")
                                for jp in range(JT // 2)]
                        def emit_score(jt):
                            jg = b * JT + jt
                            jsl = slice(b * L + jt * 128,
                                        b * L + (jt + 1) * 128)
                            ps = pds.tile([128, 512], F32, tag="ps",
                                          name=f"ps_{b}_{h}_{jt}")
                            for kk in range(2):
                                ksl = slice(2 * kk, 2 * kk + 2)
                                nc.tensor.matmul(
                                    ps, w3x8[:, ksl, jsl],
                                    x2T8[:, ksl, isl],
                                    start=(kk == 0), stop=(kk == 1),
                                    perf_mode=DR)
                            nc.scalar.activation(
                                eb2s[jt // 2][:, jt % 2, :], ps, AF.Exp,
                                bias=s2f[:, jg:jg + 1], scale=IWS)
                            if jt % 2 == 1:
                                nc.vector.tensor_tensor(
                                    eh2s[jt // 2], eb2s[jt // 2],
                                    thbc[:, None, :].broadcast_to(
                                        [128, 2, 512]),
                                    op=OP.max)

                        def emit_num(jp, b=b, pn=pn, pr16=pr16,
                                     eh2s=eh2s):
                            jg0 = b * JT + 2 * jp
                            for du in range(KU):
                                nc.tensor.matmul(
                                    pn[du],
                                    xO8[:, jg0:jg0 + 2,
                                        du * 128:(du + 1) * 128],
                                    eh2s[jp],
                                    start=(jp == 0), stop=(jp == 3),
                                    perf_mode=DR)
                            nc.tensor.matmul(pr16, ones216, eh2s[jp],
                                             start=(jp == 0), stop=(jp == 3),
                                             perf_mode=DR)

                        # previous unit's last group runs first; this
                        # unit's scores stream in behind it
                        if last_deferred is not None:
                            mults_deferred = last_deferred()
                        if unit < 4:
                            emit_fuse(8 + 2 * unit)
                            emit_fuse(8 + 2 * unit + 1)
                        for jt in range(JT):
                            emit_score(jt)
                        if mults_deferred is not None:
                            mults_deferred()
                            mults_deferred = None
                        for jp in range(JT // 2 - 1):
                            emit_num(jp)
                        last_deferred = make_lastnum(b, h, isl, pn, pr16,
                                                     emit_num)
                mults_deferred = last_deferred()
                mults_deferred()

            # ============= Phase E: fuse gates + output ====================
            with tc.tile_pool(name="pep", bufs=3, space="PSUM") as pep, \
                 tc.tile_pool(name="esb", bufs=3) as esb, \
                 tc.tile_pool(name="esx", bufs=16) as esx:
                x0ts = []
                for mt in range(NT):
                    x0t = esx.tile([128, U], F32, tag="x0t",
                                   name=f"x0t_{mt}")
                    nc.sync.dma_start(x0t, xv[mt])
                    x0ts.append(x0t)
                for mt in range(NT):
                    msl = slice(mt * 128, (mt + 1) * 128)
                    x0t = x0ts[mt]
                    pz = pep.tile([128, 512], F32, tag="pz")
                    pr2 = pep.tile([128, 512], F32, tag="pr2")
                    for kk in range(4):
                        if kk < 2:
                            lhsT = xT8[:, 2 * kk:2 * kk + 2, msl]
                        else:
                            lhsT = attT8[:, 2 * (kk - 2):2 * (kk - 2) + 2,
                                         msl]
                        wsl = slice(2 * kk, 2 * kk + 2)
                        nc.tensor.matmul(pz, lhsT, fW8[:, wsl, :],
                                         start=(kk == 0),
                                         stop=(kk == 3 and not with_bias),
                                         perf_mode=DR)
                    for k in range(KU):
                        nc.tensor.matmul(pr2, xTb[:, k, msl], rWb16[:, k, :],
                                         start=(k == 0), stop=False)
                    for kk in range(2):
                        ksl = slice(2 * kk, 2 * kk + 2)
                        nc.tensor.matmul(pr2, attT8[:, ksl, msl],
                                         rW8f[:, ksl, :],
                                         start=False,
                                         stop=(kk == 1 and not with_bias),
                                         perf_mode=DR)
                    if with_bias:
                        nc.tensor.matmul(pz, ones_row, ffb16, start=False,
                                         stop=True)
                        nc.tensor.matmul(pr2, ones_row, frb16, start=False,
                                         stop=True)
                    zh = esb.tile([128, U], BF16, tag="zh")
                    rh = esb.tile([128, U], BF16, tag="rh")
                    q = esb.tile([128, U], BF16, tag="q")
                    p2 = esb.tile([128, U], F32, tag="p2")
                    ot = esb.tile([128, U], F32, tag="ot")
                    if mt == NT - 1:
                        # last unit sets the kernel tail: shorten its
                        # serial chain by splitting across engines
                        hU = U // 2
                        nc.scalar.activation(zh, pz, AF.Sigmoid, scale=IWS)
                        nc.vector.tensor_tensor(q, zh, zh, op=OP.mult)
                        nc.scalar.activation(rh, pr2, AF.Sigmoid, scale=IWS)
                        nc.vector.tensor_tensor(p2[:, :hU], rh[:, :hU],
                                                x0t[:, :hU], op=OP.mult)
                        nc.gpsimd.tensor_tensor(p2[:, hU:], rh[:, hU:],
                                                x0t[:, hU:], op=OP.mult)
                        nc.vector.tensor_tensor(ot[:, :hU], q[:, :hU],
                                                p2[:, :hU], op=OP.add)
                        nc.gpsimd.tensor_tensor(ot[:, hU:], q[:, hU:],
                                                p2[:, hU:], op=OP.add)
                    else:
                        nc.scalar.activation(zh, pz, AF.Sigmoid, scale=IWS)
                        nc.scalar.activation(rh, pr2, AF.Sigmoid, scale=IWS)
                        nc.vector.tensor_tensor(q, zh, zh, op=OP.mult)
                        nc.vector.tensor_tensor(p2, rh, x0t, op=OP.mult)
                        nc.vector.tensor_tensor(ot, q, p2, op=OP.add)
                    nc.sync.dma_start(outv[mt], ot)

    nc.compile()
    return nc


_NC_CACHE = {}


def _get_nc(with_bias: bool = False):
    if with_bias not in _NC_CACHE:
        _NC_CACHE[with_bias] = build_nc(with_bias)
    return _NC_CACHE[with_bias]


def kernel(**inputs) -> np.ndarray:
    from concourse.bass_utils import run_bass_kernel_spmd

    full = {k: np.ascontiguousarray(np.asarray(v, dtype=np.float32))
            for k, v in inputs.items()}
    need_bias = any(np.any(full[k]) for k in ("tb", "ffb", "frb"))
    nc = _get_nc(need_bias)
    in_maps = []
    for c in range(NCORES):
        m = dict(full)
        m["inputs"] = np.ascontiguousarray(
            full["inputs"][c * BPC:(c + 1) * BPC])
        in_maps.append(m)
    res = run_bass_kernel_spmd(nc, in_maps, core_ids=list(range(NCORES)))
    return np.concatenate([res.results[c]["out"] for c in range(NCORES)],
                          axis=0)


# revision 21
# speedup vs baseline: 1.0493x; 1.0493x over previous
"""Trainium2 Bass kernel for nn_Encoding_layer (highway stack + pairwise MLP
attention + fuse gates).

Sharding: data-parallel over batch B=16 across 8 NeuronCores (2 batches per
core); all dense weights replicated. No collectives.

v3 design (mostly-fp8 matmul path):
  - fp8e4 DoubleRow GEMMs (256-contraction/instr): highway layers 0 and 1,
    attention scores/numerator/denominator, z-gate, r-gate att-half.
    fp8 weights are scaled x16 before the cast (raw 0.02-scale weights sit
    in e4m3's subnormal range); the 1/16 folds into the drain scale.
  - The r-gate inputs-half stays bf16: its error reaches the output
    un-smoothed (r multiplies raw inputs, |x0| up to ~4.5).  Attention-side
    noise only reaches the output through gate logits (~0.02 weights), so
    phase-D/x1/x2 fp8 noise is diluted ~20x.
  - s1 and s2 share one matmul (lhsT has a w1 column and a w2 column);
    the s2 row is scattered to partitions with contraction-2 matmuls
    selecting row 1 via an identity column.
  - Zero-bias fast path: tb/ffb/frb checked at runtime; fast variant drops
    phase-E bias matmuls and uses vector-relu drains.  A bias-capable
    variant compiles on demand.
  - Phase D: per unit, all 8 score matmuls are emitted before the
    numerator accumulations so the in-order PE queue never waits on the
    exp/clamp chain.  th/1-per-unit reciprocal broadcasts ride idle DMA
    engines (partition-stride-0 source APs) instead of PE matmuls; pn
    psum banks drain via a fused multiply (psum x 1/den -> fp8 attT8)
    deferred to the start of the next unit.

Per-core layouts (n = 2 batches x L=1024 = 2048 token-columns):
  xT8/x1T8/x2T8/w3x8/attT8 (fp8), xTb (bf16): [128, 4, 2048]
    [u mod 128, u div 128, n]
  xO8 (fp8): [128, 16, 512] row-major x2 (lhsT for the attention numerator)
  Attention: S^T[j,i] = s3[j,i] (PE, w3*x^T as lhsT) + s2[j] (ACT exp bias).
  The per-column term s1[i]+ab never enters the matmuls: a per-column
  factor cancels in the softmax, so relu becomes a clamp against
  th[i] = exp(-(s1[i]+ab)).
"""

import numpy as np

B, L, U, H = 16, 1024, 512, 2
NCORES = 8
BPC = B // NCORES          # batches per core
N = BPC * L                # token columns per core
KU = U // 128              # 4  u-tiles
NT = N // 128              # 16 row-tiles per core
NS = N // 512              # 4  512-wide column slices per core
JT = L // 128              # 8  j-tiles per batch
IH = L // 512              # 2  i-halves per batch
WS = 16.0                  # fp8 weight scale
IWS = 1.0 / WS


def build_nc(with_bias: bool):
    import concourse.bacc as bacc
    import concourse.tile as tile
    from concourse import mybir
    from concourse.masks import make_identity

    F32 = mybir.dt.float32
    BF16 = mybir.dt.bfloat16
    FP8 = mybir.dt.float8e4
    AF = mybir.ActivationFunctionType
    OP = mybir.AluOpType
    DR = mybir.MatmulPerfMode.DoubleRow

    nc = bacc.Bacc("TRN2", target_bir_lowering=False, debug=False,
                   num_devices=NCORES)

    x_in = nc.dram_tensor("inputs", [BPC, L, U], F32, kind="ExternalInput").ap()
    tW = nc.dram_tensor("tW", [H, U, U], F32, kind="ExternalInput").ap()
    tb = nc.dram_tensor("tb", [H, U], F32, kind="ExternalInput").ap()
    cW = nc.dram_tensor("cW", [H, U, U], F32, kind="ExternalInput").ap()
    cb = nc.dram_tensor("cb", [H, U], F32, kind="ExternalInput").ap()
    aW = nc.dram_tensor("aW", [3 * U], F32, kind="ExternalInput").ap()
    ab = nc.dram_tensor("ab", [1], F32, kind="ExternalInput").ap()
    frW = nc.dram_tensor("frW", [2 * U, U], F32, kind="ExternalInput").ap()
    frb = nc.dram_tensor("frb", [U], F32, kind="ExternalInput").ap()
    ffW = nc.dram_tensor("ffW", [2 * U, U], F32, kind="ExternalInput").ap()
    ffb = nc.dram_tensor("ffb", [U], F32, kind="ExternalInput").ap()
    out = nc.dram_tensor("out", [BPC, L, U], F32, kind="ExternalOutput").ap()

    xv = x_in.flatten_outer_dims().rearrange("(t p) u -> t p u", p=128)
    outv = out.flatten_outer_dims().rearrange("(t p) u -> t p u", p=128)

    fWv = ffW.rearrange("(k p) m -> k p m", p=128)
    rWv = frW.rearrange("(k p) m -> k p m", p=128)

    with tile.TileContext(nc) as tc:
        with tc.tile_pool(name="pers", bufs=1) as pers, \
             tc.tile_pool(name="dram", bufs=1, space="DRAM") as dpool:
            # ---- persistent SBUF tensors ----
            xT8 = pers.tile([128, KU, N], FP8, tag="xT8")      # inputs^T
            xTb = pers.tile([128, KU, N], BF16, tag="xTb")     # inputs^T bf16
            x1T8 = pers.tile([128, KU, N], FP8, tag="x1T8")
            x2T8 = pers.tile([128, KU, N], FP8, tag="x2T8")
            w3x8 = pers.tile([128, KU, N], FP8, tag="w3x8")
            attT8 = pers.tile([128, KU, N], FP8, tag="attT8")
            xO8 = pers.tile([128, NT, U], FP8, tag="xO8")
            tW8 = pers.tile([128, KU, U], FP8, tag="tW8")      # layer0, x16
            cW8 = pers.tile([128, KU, U], FP8, tag="cW8")      # layer0, x16
            tW8b = pers.tile([128, KU, U], FP8, tag="tW8b")    # layer1, x16
            cW8b = pers.tile([128, KU, U], FP8, tag="cW8b")    # layer1, x16
            fW8 = pers.tile([128, 2 * KU, U], FP8, tag="fW8")  # x16
            rWb16 = pers.tile([128, KU, U], BF16, tag="rWb16")  # x16 inputs
            rW8f = pers.tile([128, KU, U], FP8, tag="rW8f")    # x16 att-half
            tbsb = pers.tile([128, H, KU], F32, tag="tbsb")
            cbsb = pers.tile([128, H, KU], F32, tag="cbsb")
            awsb = pers.tile([128, 12], F32, tag="awsb")       # w1|w2|w3 cols
            aw3 = pers.tile([128, KU], F32, tag="aw3")         # 16*w3
            w12h8 = pers.tile([128, KU, 16], FP8, tag="w12h8")  # x16 w1,w2,0.
            ab_sb = pers.tile([1, 1], F32, tag="ab_sb")
            nab_sb = pers.tile([1, 1], F32, tag="nab_sb")
            ffb16 = pers.tile([1, U], BF16, tag="ffb16")       # x16
            frb16 = pers.tile([1, U], BF16, tag="frb16")       # x16
            thr = pers.tile([1, N], BF16, tag="thr")           # exp(-(s1+ab))
            thrd = dpool.tile([1, N], BF16, tag="thrd")        # DRAM copy
            s2f = pers.tile([128, NT], F32, tag="s2f")
            s12sb = pers.tile([2, N], BF16, tag="s12sb")
            ones_row = pers.tile([1, 128], BF16, tag="ones_row")
            ones216 = pers.tile([128, 2, 16], FP8, tag="ones216")
            identf = pers.tile([128, 128], F32, tag="identf")
            ident8 = pers.tile([128, 128], FP8, tag="ident8")
            identb = pers.tile([128, 128], BF16, tag="identb")

            nc.vector.memset(ones_row, 1.0)
            nc.vector.memset(ones216, 1.0)
            make_identity(nc, identf)
            make_identity(nc, ident8)
            make_identity(nc, identb)

            # fuse-gate weight chunks dripped through phases B+C and D
            fuse_chunks = (
                [(fWv, fW8, k, k) for k in range(2 * KU)] +
                [(rWv, rWb16, k, k) for k in range(KU)] +
                [(rWv, rW8f, k, k - KU) for k in range(KU, 2 * KU)])

            def emit_fuse(ci):
                wv_, wdst_, ksrc_, kdst_ = fuse_chunks[ci]
                wsf = pers.tile([128, U], F32, tag="wsf", bufs=4,
                                name=f"wsf_{ci}")
                nc.sync.dma_start(wsf, wv_[ksrc_])
                if ci % 2 == 0:
                    nc.vector.tensor_scalar_mul(wdst_[:, kdst_, :], wsf, WS)
                else:
                    nc.scalar.mul(wdst_[:, kdst_, :], wsf, WS)

            # ======== Phase A: loads, transpose, highway layers ==========
            with tc.tile_pool(name="stg", bufs=3) as stg, \
                 tc.tile_pool(name="stgx", bufs=8) as stgx, \
                 tc.tile_pool(name="stgw", bufs=8) as stgw, \
                 tc.tile_pool(name="transP", bufs=2, space="PSUM") as transP, \
                 tc.tile_pool(name="hw0P", bufs=3, space="PSUM") as hw0P:
                def emit_weights(l, wi):
                    wsrc = (tW, cW)[wi]
                    wdst = ((tW8, cW8), (tW8b, cW8b))[l][wi]
                    wv = wsrc[l].rearrange("(k p) m -> k p m", p=128)
                    for k in range(KU):
                        ws = stgw.tile([128, U], F32, tag="ws",
                                       name=f"ws_{l}_{wi}_{k}")
                        nc.sync.dma_start(ws, wv[k])
                        if k % 2 == 0:
                            nc.vector.tensor_scalar_mul(wdst[:, k, :], ws, WS)
                        else:
                            nc.scalar.mul(wdst[:, k, :], ws, WS)

                # small tensors first (layer-0 needs biases)
                nc.sync.dma_start(
                    tbsb, tb.rearrange("l (m p) -> p l m", p=128))
                nc.sync.dma_start(
                    cbsb, cb.rearrange("l (m p) -> p l m", p=128))
                nc.sync.dma_start(
                    awsb, aW.rearrange("(w m p) -> p (w m)", p=128, w=3))
                nc.vector.memset(w12h8, 0.0)
                nc.vector.tensor_scalar_mul(w12h8[:, :, 0], awsb[:, 0:KU], WS)
                nc.vector.tensor_scalar_mul(
                    w12h8[:, :, 1], awsb[:, KU:2 * KU], WS)
                nc.vector.tensor_scalar_mul(aw3, awsb[:, 2 * KU:3 * KU], WS)
                nc.sync.dma_start(ab_sb, ab[None, :])
                nc.scalar.mul(nab_sb, ab_sb, -1.0)
                fb = stg.tile([1, U], F32, tag="fb")
                nc.sync.dma_start(fb, ffb[None, :])
                nc.scalar.mul(ffb16, fb, WS)
                fb2 = stg.tile([1, U], F32, tag="fb")
                nc.sync.dma_start(fb2, frb[None, :])
                nc.scalar.mul(frb16, fb2, WS)

                # warm the PE p-state during the initial DMA wait
                wpt = hw0P.tile([128, 512], F32, tag="pt", name="warm_pt")
                wpc = hw0P.tile([128, 512], F32, tag="pc", name="warm_pc")
                for i in range(24):
                    nc.tensor.matmul((wpt, wpc)[i % 2][:, 0:128],
                                     identf, identf,
                                     start=True, stop=True)

                def trans_block(tg, tt):
                    t = tg * 4 + tt
                    xs = stgx.tile([128, U], F32, tag="xs",
                                   name=f"xs_{t}")
                    nc.sync.dma_start(xs, xv[t])
                    ptt = transP.tile([128, 512], F32, tag="ptt",
                                      name=f"ptt_{t}")
                    for k in range(KU):
                        nc.tensor.transpose(
                            ptt[:, k * 128:(k + 1) * 128],
                            xs[:, k * 128:(k + 1) * 128], identf)
                    csl = slice(tg * 512 + tt * 128,
                                tg * 512 + (tt + 1) * 128)
                    pv = ptt.rearrange("p (k c) -> p k c", k=KU)
                    nc.scalar.copy(xTb[:, :, csl], pv)

                def trans_group(tg):
                    for tt in range(4):
                        trans_block(tg, tt)
                    nsl = slice(tg * 512, (tg + 1) * 512)
                    nc.vector.tensor_copy(xT8[:, :, nsl], xTb[:, :, nsl])

                def hw_layer(l, tg):
                    nsl = slice(tg * 512, (tg + 1) * 512)
                    xmm = xT8 if l == 0 else x1T8
                    xew = xTb[:, :, nsl] if l == 0 else x1T8[:, :, nsl]
                    wT, wC = ((tW8, cW8), (tW8b, cW8b))[l]
                    th = stg.tile([128, KU, 512], BF16, tag="th",
                                  name=f"th{l}_{tg}")
                    ch = stg.tile([128, KU, 512], BF16, tag="ch",
                                  name=f"ch{l}_{tg}")
                    for m in range(KU):
                        pt = hw0P.tile([128, 512], F32, tag="pt",
                                       name=f"pt{l}_{tg}_{m}")
                        pc = hw0P.tile([128, 512], F32, tag="pc",
                                       name=f"pc{l}_{tg}_{m}")
                        for kk in range(2):
                            ksl = slice(2 * kk, 2 * kk + 2)
                            nc.tensor.matmul(
                                pt, wT[:, ksl, m * 128:(m + 1) * 128],
                                xmm[:, ksl, nsl],
                                start=(kk == 0), stop=(kk == 1),
                                perf_mode=DR)
                        for kk in range(2):
                            ksl = slice(2 * kk, 2 * kk + 2)
                            nc.tensor.matmul(
                                pc, wC[:, ksl, m * 128:(m + 1) * 128],
                                xmm[:, ksl, nsl],
                                start=(kk == 0), stop=(kk == 1),
                                perf_mode=DR)
                        # drains: relu vector (m 0,1) / scalar (m 2,3);
                        # sigmoid always on scalar (ACT-only func)
                        if with_bias or m >= 2:
                            nc.scalar.activation(
                                th[:, m, :], pt, AF.Relu,
                                bias=tbsb[:, l, m:m + 1], scale=IWS)
                        else:
                            nc.vector.tensor_scalar(
                                th[:, m, :], pt, 0.0, IWS,
                                op0=OP.max, op1=OP.mult)
                        nc.scalar.activation(
                            ch[:, m, :], pc, AF.Sigmoid,
                            bias=cbsb[:, l, m:m + 1], scale=IWS)
                    # slice-wide elementwise: x_out = xin + c*(t - xin)
                    dh = stg.tile([128, KU, 512], BF16, tag="dh",
                                  name=f"dh{l}_{tg}")
                    nc.vector.tensor_tensor(dh, th, xew, op=OP.subtract)
                    mh = stg.tile([128, KU, 512], BF16, tag="mh",
                                  name=f"mh{l}_{tg}")
                    nc.vector.tensor_tensor(mh[:, 0:3, :], ch[:, 0:3, :],
                                            dh[:, 0:3, :], op=OP.mult)
                    nc.gpsimd.tensor_tensor(mh[:, 3, :], ch[:, 3, :],
                                            dh[:, 3, :], op=OP.mult)
                    dst = x1T8 if l == 0 else x2T8
                    nc.vector.tensor_tensor(
                        dst[:, :, nsl], xew, mh, op=OP.add)

                trans_group(0)
                emit_weights(0, 0)
                emit_weights(0, 1)

                # 3-stream pipeline: T(s) | L0(s-1) | L1(s-2)
                for s in range(1, NS + 2):
                    if s - 1 < NS:
                        hw_layer(0, s - 1)
                    if s < NS:
                        trans_group(s)
                    if 0 <= s - 2:
                        hw_layer(1, s - 2)
                    if s == 1:
                        emit_weights(1, 0)
                        emit_weights(1, 1)

            # ===== Phase B+C: attention prep (xO8 transposes, w3x, s1,
            # s2 via merged matmul + partition scatter) ============
            with tc.tile_pool(name="xop", bufs=2, space="PSUM") as xop, \
                 tc.tile_pool(name="pc1", bufs=1, space="PSUM") as pc1:
                ps12 = pc1.tile([16, 512], F32, tag="ps12")
                s2p = pc1.tile([128, NT], F32, tag="s2p")

                def prep_block(t):
                    nsl = slice(t * 512, (t + 1) * 512)
                    for k in range(KU):
                        nc.vector.tensor_scalar_mul(
                            w3x8[:, k, nsl], x2T8[:, k, nsl],
                            aw3[:, k:k + 1])
                    for jt in range(4 * t, 4 * t + 4):
                        ptr = xop.tile([128, 512, 2], FP8, tag="ptr",
                                       name=f"ptr_{jt}")
                        for k in range(KU):
                            nc.tensor.transpose(
                                ptr[:, k * 128:(k + 1) * 128, 0],
                                x2T8[:, k, jt * 128:(jt + 1) * 128],
                                ident8)
                        if jt % 2 == 0:
                            nc.vector.tensor_copy(
                                xO8[:, jt, :], ptr[:, :, 0])
                        else:
                            nc.scalar.copy(xO8[:, jt, :], ptr[:, :, 0])
                    # merged s1/s2: out row0 = x2.w1, row1 = x2.w2
                    for kk in range(2):
                        ksl = slice(2 * kk, 2 * kk + 2)
                        nc.tensor.matmul(ps12, w12h8[:, ksl, :],
                                         x2T8[:, ksl, nsl],
                                         start=(kk == 0), stop=(kk == 1),
                                         perf_mode=DR)
                    nc.scalar.activation(
                        thr[:, nsl], ps12[0:1, :], AF.Exp,
                        bias=nab_sb, scale=-IWS)
                    nc.sync.dma_start(thrd[:, nsl], thr[:, nsl])
                    nc.vector.tensor_copy(s12sb[:, nsl], ps12[0:2, :])
                    # scatter s2 row chunks onto partitions: contract-2
                    # matmul whose rhs (identity col 1) selects row 1
                    for jt in range(4 * t, 4 * t + 4):
                        nc.tensor.matmul(
                            s2p[:, jt:jt + 1],
                            s12sb[:, jt * 128:(jt + 1) * 128],
                            identb[0:2, 1:2], start=True, stop=True)
                    emit_fuse(2 * t)
                    emit_fuse(2 * t + 1)

                for t in range(NS):
                    prep_block(t)
                nc.scalar.mul(s2f, s2p, IWS)

            # ============= Phase D: pairwise softmax attention =============
            with tc.tile_pool(name="pdn", bufs=1, space="PSUM") as pdn, \
                 tc.tile_pool(name="pds", bufs=3, space="PSUM") as pds, \
                 tc.tile_pool(name="prp", bufs=1, space="PSUM") as prp, \
                 tc.tile_pool(name="dsb", bufs=4) as dsb:
                # th-clamp broadcasts ride idle DMA engines, prefetched
                # for all 8 units up front (partition-stride-0 source)
                thbcs = []
                for b in range(BPC):
                    for h in range(IH):
                        isl = slice(b * L + h * 512, b * L + (h + 1) * 512)
                        thbc = dsb.tile([128, 512], BF16, tag="thbc",
                                        bufs=8, name=f"thbc_{b}_{h}")
                        nc.sync.dma_start(
                            thbc, thrd[0:1, isl].broadcast_to([128, 512]))
                        thbcs.append(thbc)

                def make_tail(b, h, isl, rec, pn):
                    def tail():
                        recd = dpool.tile([1, 512], F32, tag="recd",
                                          bufs=2, name=f"recd_{b}_{h}")
                        nc.sync.dma_start(recd, rec)
                        rbc = dsb.tile([128, 512], F32, tag="rbc",
                                       bufs=2, name=f"rbc_{b}_{h}")
                        nc.sync.dma_start(
                            rbc, recd[0:1, :].broadcast_to([128, 512]))
                        for du in range(KU):
                            nc.vector.tensor_tensor(
                                attT8[:, du, isl], pn[du], rbc,
                                op=OP.mult)
                    return tail

                deferred = None
                for b in range(BPC):
                    for h in range(IH):
                        unit = b * IH + h
                        thbc = thbcs[unit]
                        if unit < 4:
                            emit_fuse(8 + 2 * unit)
                            emit_fuse(8 + 2 * unit + 1)
                        isl = slice(b * L + h * 512, b * L + (h + 1) * 512)
                        # previous unit's normalize tail first: its pn
                        # banks must free before this unit's numerator
                        if deferred is not None:
                            deferred()
                        pn = [pdn.tile([128, 512], F32, tag=f"pn{du}",
                                       name=f"pn_{b}_{h}_{du}")
                              for du in range(KU)]
                        pr16 = prp.tile([16, 512], F32, tag="pr16",
                                        name=f"pr16_{b}_{h}")
                        eh2s = [dsb.tile([128, 2, 512], FP8, tag="eh2",
                                         name=f"eh2_{b}_{h}_{jp}")
                                for jp in range(JT // 2)]
                        eb2s = [dsb.tile([128, 2, 512], BF16, tag="eb2",
                                         bufs=3, name=f"eb2_{b}_{h}_{jp}")
                                for jp in range(JT // 2)]

                        def emit_score(jt):
                            jg = b * JT + jt
                            jsl = slice(b * L + jt * 128,
                                        b * L + (jt + 1) * 128)
                            ps = pds.tile([128, 512], F32, tag="ps",
                                          name=f"ps_{b}_{h}_{jt}")
                            for kk in range(2):
                                ksl = slice(2 * kk, 2 * kk + 2)
                                nc.tensor.matmul(
                                    ps, w3x8[:, ksl, jsl],
                                    x2T8[:, ksl, isl],
                                    start=(kk == 0), stop=(kk == 1),
                                    perf_mode=DR)
                            nc.scalar.activation(
                                eb2s[jt // 2][:, jt % 2, :], ps, AF.Exp,
                                bias=s2f[:, jg:jg + 1], scale=IWS)
                            if jt % 2 == 1:
                                nc.vector.tensor_tensor(
                                    eh2s[jt // 2], eb2s[jt // 2],
                                    thbc[:, None, :].broadcast_to(
                                        [128, 2, 512]),
                                    op=OP.max)

                        def emit_num(jp):
                            jg0 = b * JT + 2 * jp
                            for du in range(KU):
                                nc.tensor.matmul(
                                    pn[du],
                                    xO8[:, jg0:jg0 + 2,
                                        du * 128:(du + 1) * 128],
                                    eh2s[jp],
                                    start=(jp == 0), stop=(jp == 3),
                                    perf_mode=DR)
                            nc.tensor.matmul(pr16, ones216, eh2s[jp],
                                             start=(jp == 0), stop=(jp == 3),
                                             perf_mode=DR)

                        for jt in range(JT):
                            emit_score(jt)
                        for jp in range(JT // 2):
                            emit_num(jp)
                        rec = dsb.tile([1, 512], F32, tag="rec",
                                       name=f"rec_{b}_{h}")
                        nc.vector.reciprocal_approx_fast(rec, pr16[0:1, :])
                        deferred = make_tail(b, h, isl, rec, pn)
                deferred()

            # ============= Phase E: fuse gates + output ====================
            with tc.tile_pool(name="pep", bufs=3, space="PSUM") as pep, \
                 tc.tile_pool(name="esb", bufs=3) as esb, \
                 tc.tile_pool(name="esx", bufs=16) as esx:
                x0ts = []
                for mt in range(NT):
                    x0t = esx.tile([128, U], F32, tag="x0t",
                                   name=f"x0t_{mt}")
                    nc.sync.dma_start(x0t, xv[mt])
                    x0ts.append(x0t)
                for mt in range(NT):
                    msl = slice(mt * 128, (mt + 1) * 128)
                    x0t = x0ts[mt]
                    pz = pep.tile([128, 512], F32, tag="pz")
                    pr2 = pep.tile([128, 512], F32, tag="pr2")
                    for kk in range(4):
                        if kk < 2:
                            lhsT = xT8[:, 2 * kk:2 * kk + 2, msl]
                        else:
                            lhsT = attT8[:, 2 * (kk - 2):2 * (kk - 2) + 2,
                                         msl]
                        wsl = slice(2 * kk, 2 * kk + 2)
                        nc.tensor.matmul(pz, lhsT, fW8[:, wsl, :],
                                         start=(kk == 0),
                                         stop=(kk == 3 and not with_bias),
                                         perf_mode=DR)
                    for k in range(KU):
                        nc.tensor.matmul(pr2, xTb[:, k, msl], rWb16[:, k, :],
                                         start=(k == 0), stop=False)
                    for kk in range(2):
                        ksl = slice(2 * kk, 2 * kk + 2)
                        nc.tensor.matmul(pr2, attT8[:, ksl, msl],
                                         rW8f[:, ksl, :],
                                         start=False,
                                         stop=(kk == 1 and not with_bias),
                                         perf_mode=DR)
                    if with_bias:
                        nc.tensor.matmul(pz, ones_row, ffb16, start=False,
                                         stop=True)
                        nc.tensor.matmul(pr2, ones_row, frb16, start=False,
                                         stop=True)
                    zh = esb.tile([128, U], BF16, tag="zh")
                    rh = esb.tile([128, U], BF16, tag="rh")
                    q = esb.tile([128, U], BF16, tag="q")
                    p2 = esb.tile([128, U], F32, tag="p2")
                    ot = esb.tile([128, U], F32, tag="ot")
                    if mt == NT - 1:
                        # last unit sets the kernel tail: shorten its
                        # serial chain by splitting across engines
                        hU = U // 2
                        nc.scalar.activation(zh, pz, AF.Sigmoid, scale=IWS)
                        nc.vector.tensor_tensor(q, zh, zh, op=OP.mult)
                        nc.scalar.activation(rh, pr2, AF.Sigmoid, scale=IWS)
                        nc.vector.tensor_tensor(p2[:, :hU], rh[:, :hU],
                                                x0t[:, :hU], op=OP.mult)
                        nc.gpsimd.tensor_tensor(p2[:, hU:], rh[:, hU:],
                                                x0t[:, hU:], op=OP.mult)
                        nc.vector.tensor_tensor(ot[:, :hU], q[:, :hU],
                                                p2[:, :hU], op=OP.add)
                        nc.gpsimd.tensor_tensor(ot[:, hU:], q[:, hU:],
                                                p2[:, hU:], op=OP.add)
                    else:
                        nc.scalar.activation(zh, pz, AF.Sigmoid, scale=IWS)
                        nc.scalar.activation(rh, pr2, AF.Sigmoid, scale=IWS)
                        nc.vector.tensor_tensor(q, zh, zh, op=OP.mult)
                        nc.vector.tensor_tensor(p2, rh, x0t, op=OP.mult)
                        nc.vector.tensor_tensor(ot, q, p2, op=OP.add)
                    nc.sync.dma_start(outv[mt], ot)

    nc.compile()
    return nc


_NC_CACHE = {}


def _get_nc(with_bias: bool = False):
    if with_bias not in _NC_CACHE:
        _NC_CACHE[with_bias] = build_nc(with_bias)
    return _NC_CACHE[with_bias]


def kernel(**inputs) -> np.ndarray:
    from concourse.bass_utils import run_bass_kernel_spmd

    full = {k: np.ascontiguousarray(np.asarray(v, dtype=np.float32))
            for k, v in inputs.items()}
    need_bias = any(np.any(full[k]) for k in ("tb", "ffb", "frb"))
    nc = _get_nc(need_bias)
    in_maps = []
    for c in range(NCORES):
        m = dict(full)
        m["inputs"] = np.ascontiguousarray(
            full["inputs"][c * BPC:(c + 1) * BPC])
        in_maps.append(m)
    res = run_bass_kernel_spmd(nc, in_maps, core_ids=list(range(NCORES)))
    return np.concatenate([res.results[c]["out"] for c in range(NCORES)],
                          axis=0)
